# revision 1
# baseline (speedup 1.0000x reference)
"""Combine-STFT interleave kernel for Trainium2 (8 NeuronCores, SPMD).

Problem: X [8, 16, 513, 1024] f32, channel pairs (2c, 2c+1) = (real, imag).
Output: complex64 [8, 8, 513, 1024] == f32 [..., 2] with interleaved (r, i)
pairs.  Pure memory reshuffle, HBM-bandwidth bound.

Sharding: batch dim across the 8 cores (no communication).  Per core:
one DMA loads a (real, imag) chunk pair into SBUF (contiguous rows), the
DVE interleaves them with two strided copies, one DMA stores the
interleaved tile back contiguously.  Raw Bass with explicit single-sem
waits (this walrus build rejects instructions with >1 sync-wait, which
rules out the Tile scheduler).  In-DMAs issue from SP's HWDGE ring and
out-DMAs from ACT's, so load and store streams overlap.
"""

import os
import sys

for _p in ("/opt/trn_rl_repo", "/root/.axon_site/_ro/trn_rl_repo"):
    if os.path.isdir(_p) and _p not in sys.path:
        sys.path.insert(0, _p)

import numpy as np

import concourse.bass as bass
import concourse.mybir as mybir
from concourse.bass_utils import run_bass_kernel_spmd

N_CORES = 8
B, D, NRTF, NSEG = 8, 16, 513, 1024
NCH = D // 2                 # complex channels per batch
PLANE = NRTF * NSEG          # 525312 = 128 * 4104
P = 128
CHUNKS = 2                   # chunks per plane
F = PLANE // (P * CHUNKS)    # free-dim elements per chunk row (2052)
NITER = NCH * CHUNKS
NBUF = 4

_nc_cache = None


def _build(chunks=CHUNKS, nbuf_t=NBUF, nbuf_o=None, merge_in=False):
    from contextlib import ExitStack

    if nbuf_o is None:
        nbuf_o = nbuf_t
    if merge_in:
        assert chunks == 2 and nbuf_t % 2 == 0
    f32 = mybir.dt.float32
    F = PLANE // (P * chunks)
    NITER = NCH * chunks
    nc = bass.Bass()
    X = nc.declare_dram_parameter("X", [D, chunks, P, F], f32, isOutput=False)
    Y = nc.declare_dram_parameter("Y", [NCH, chunks, P, 2 * F], f32, isOutput=True)

    W = 2 * F  # slot width: one (real, imag) chunk pair

    # Per-slot DMA-completion sems.  A shared cumulative sem (wait >= 16*(i+1))
    # is unsound: the 16 increments per DMA come from 16 independent SDMA
    # engines, so under engine skew the sum can pass the threshold while a
    # slow engine still owes data for iteration i.  Per-slot sems close that
    # hole — an early increment could only come from a future DMA to the same
    # slot, which the pipeline's own waits make impossible.
    with ExitStack() as ctx:
        T = ctx.enter_context(nc.sbuf_tensor([P, nbuf_t * W], f32))
        O = ctx.enter_context(nc.sbuf_tensor([P, nbuf_o * W], f32))
        s_in = [
            ctx.enter_context(nc.semaphore(f"s_in{j}")) for j in range(nbuf_t)
        ]
        s_out = [
            ctx.enter_context(nc.semaphore(f"s_out{j}")) for j in range(nbuf_o)
        ]
        s_dve = ctx.enter_context(nc.semaphore("s_dve"))
        block = ctx.enter_context(nc.Block())

        def src_pair(it):
            ch, k = divmod(it, chunks)
            return X[2 * ch : 2 * ch + 2, k].rearrange("two p f -> p two f")

        def dst_chunk(it):
            ch, k = divmod(it, chunks)
            return Y[ch, k]

        @block.sync
        def _(sp):
            if merge_in:
                # One 4D-AP DMA per channel fills two adjacent slots with
                # both (real, imag) chunk pairs; s_in is indexed by slot-pair.
                for j in range(NITER // 2):
                    i1 = 2 * j + 1
                    s0 = (2 * j) % nbuf_t
                    if i1 >= nbuf_t:
                        sp.wait_ge(s_dve, i1 - nbuf_t + 1)
                    dst = T[:, s0 * W : (s0 + 2) * W].rearrange(
                        "p (k two f) -> p k two f", k=2, two=2
                    )
                    src = X[2 * j : 2 * j + 2].rearrange("two k p f -> p k two f")
                    sp.dma_start(out=dst, in_=src).then_inc(s_in[s0 // 2], 16)
            else:
                for i in range(NITER):
                    slot = i % nbuf_t
                    if i >= nbuf_t:
                        sp.wait_ge(s_dve, i - nbuf_t + 1)
                    dst = T[:, slot * W : (slot + 1) * W].rearrange(
                        "p (two f) -> p two f", two=2
                    )
                    sp.dma_start(out=dst, in_=src_pair(i)).then_inc(s_in[slot], 16)

        @block.vector
        def _(v):
            for i in range(NITER):
                slot_t, gen_t = i % nbuf_t, i // nbuf_t
                slot_o, gen_o = i % nbuf_o, i // nbuf_o
                if merge_in:
                    v.wait_ge(s_in[slot_t // 2], 16 * (gen_t + 1))
                else:
                    v.wait_ge(s_in[slot_t], 16 * (gen_t + 1))
                if i >= nbuf_o:
                    v.wait_ge(s_out[slot_o], 16 * gen_o)
                tt = T[:, slot_t * W : (slot_t + 1) * W]
                ot = O[:, slot_o * W : (slot_o + 1) * W]
                nc.vector.tensor_copy(out=ot[:, 0::2], in_=tt[:, 0:F])
                nc.vector.tensor_copy(out=ot[:, 1::2], in_=tt[:, F : 2 * F]).then_inc(
                    s_dve, 1
                )

        @block.scalar
        def _(act):
            for i in range(NITER):
                slot_o = i % nbuf_o
                act.wait_ge(s_dve, i + 1)
                act.dma_start(
                    out=dst_chunk(i), in_=O[:, slot_o * W : (slot_o + 1) * W]
                ).then_inc(s_out[slot_o], 16)
            last_gen = {}
            for i in range(NITER):
                last_gen[i % nbuf_o] = i // nbuf_o + 1
            for j, g in last_gen.items():
                act.wait_ge(s_out[j], 16 * g)

    return nc


def _get_nc(chunks=CHUNKS, nbuf_t=NBUF, nbuf_o=None, merge_in=False):
    global _nc_cache
    key = (chunks, nbuf_t, nbuf_o, merge_in)
    if _nc_cache is None or _nc_cache[0] != key:
        _nc_cache = (key, _build(chunks, nbuf_t, nbuf_o, merge_in))
    return _nc_cache[1]


def _run(X, chunks=CHUNKS, nbuf_t=NBUF, nbuf_o=None, merge_in=False, **kwargs):
    X = np.ascontiguousarray(X, dtype=np.float32)
    f = PLANE // (P * chunks)
    in_maps = [{"X": X[b].reshape(D, chunks, P, f)} for b in range(N_CORES)]
    return run_bass_kernel_spmd(
        _get_nc(chunks, nbuf_t, nbuf_o, merge_in),
        in_maps,
        list(range(N_CORES)),
        **kwargs,
    )


def _unshard(results):
    out = np.empty((B, NCH, NRTF, NSEG), dtype=np.complex64)
    for b in range(N_CORES):
        y = np.ascontiguousarray(results[b]["Y"], dtype=np.float32)
        out[b] = y.reshape(NCH, 2 * PLANE).view(np.complex64).reshape(NCH, NRTF, NSEG)
    return out


def kernel(X: np.ndarray) -> np.ndarray:
    return _unshard(_run(X).results)


def kernel_traced(X: np.ndarray):
    """Returns (output, BassKernelResults) with hardware trace enabled."""
    res = _run(X, trace=True)
    return _unshard(res.results), res

